# revision 2
# baseline (speedup 1.0000x reference)
"""Trainium2 Bass kernel for classical self-attention (B=4, N=4096, D=1024, fp32).

  q = x @ Wq.T ; k = x @ Wk.T
  out = softmax(q @ k.T / sqrt(D)) @ x

Sharding: 8 cores = (batch b = c//2) x (query half h = c%2, 2048 queries each).
Each core holds all 4096 keys of its batch, so softmax rows are core-local and
no collectives are needed.

v2 algorithm — fuses the K projection away via
  scores = q k^T = x Wq^T Wk x^T = (x_q M) x^T  with  M = Wq^T Wk  [D,D]:

  phase M: M = Wq^T @ Wk on-chip (contraction over PE partitions with both
           weights in natural layout) — 128 matmuls, replaces the entire
           4096-key K projection (512 matmuls) of v1.
  phase Q: q'T = (x_q M)^T directly in transposed layout [D, NQ], SBUF
           resident.
  per 512-query super-block:
    phase A: transposed score chunks pT[k, q] with SBUF-RESIDENT x^T slices
             as stationary (x^T is 8MB in bf16 and never re-read from HBM),
             exp on ScalarE straight PSUM->SBUF (scale=1/32 folded in; no
             max-subtraction: scores ~ N(0,1.x), fp32 exp is exact-safe).
    phase B: out accumulated over 32 key chunks in two 4-bank PSUM groups
             (d-halves), with pT chunks as stationary; softmax denominators
             come from free-size-1 matmuls that REUSE the same pT stationary
             against a ones vector, landing s directly in [128q, 1] PSUM
             layout (no ones-vector row-sum passes, no DRAM reshape
             roundtrip); normalization by 1/s at PSUM eviction.

All matmuls run in bf16 (full PE rate; measured end-to-end error vs the fp32
reference ~3e-3, threshold 2e-2). PSUM accumulation is fp32 throughout.

This file also carries two workarounds for this container's walrus build,
which rejects any instruction carrying more than one sync wait.
"""

import re

import numpy as np

import bass_rust
import concourse.bass as bass
import concourse.mybir as mybir
from concourse.tile import TileContext

B, N, D = 4, 4096, 1024
NQ = N // 2          # queries per core
QS = 512             # query super-block
P = 128              # partitions
DC = D // P          # contraction chunks (8)
NCH = N // P         # key chunks (32)
NQS = NQ // QS       # query super-blocks (4)
NSUB = QS // P       # 128-query sub-blocks per super-block (4)
F32 = mybir.dt.float32
BF16 = mybir.dt.bfloat16
EXP = mybir.ActivationFunctionType.Exp
SCALE = 1.0 / 32.0   # 1/sqrt(D)
N_CORES = 8


class SplitDrainTileContext(TileContext):
    """The TileContext exit emits one SP Drain waiting on every proc's final
    semaphore value; this walrus build allows a single sync wait per
    instruction.  Emit the waits as single-wait NOPs first, then a drain
    that needs no waits of its own."""

    def _drain_and_barrier(self, tick_clock, wait_clock):
        gc = tick_clock.global_clock
        ticks = [int(s) for s in re.findall(r"\d+", repr(gc))]
        for proc, t in enumerate(ticks):
            if t > 0:
                single = bass_rust.VectorClock()
                single.require_at_least(proc, t)
                nop = self.nc.sync.nop(nofuse=True, hint="split_drain_wait")
                wait_clock.add_sem_waits(nop.ins, bass_rust.ScopedClock({None: single}))
        drain_inst = self.nc.sync.drain()
        wait_clock.add_sem_waits(
            drain_inst.ins,
            bass_rust.ScopedClock({None: gc}),
            bass_rust.ScopedClock({None: gc.copy()}),
        )
        self.nc.all_engine_barrier()
        assert self.sems is not None
        popped = self.nc._tile_sem_poison_stack.pop()
        assert popped is self._sem_poison
        self.nc.clear_and_free_semaphores(list(self.sems.allocated().values()))
        self.nc.all_engine_barrier()


def _split_multiwaits(nc: bass.Bass, max_waits: int = 1) -> None:
    """Hoist extra sync waits onto injected NoOps placed immediately before
    the instruction in the same basic block (engines execute their stream in
    bb order, so the engine blocks on each NoOp's wait before reaching the
    real instruction)."""
    ctr = 0
    for bb in nc.main_func.blocks:
        new_list = []
        changed = False
        for inst in bb.instructions:
            si = inst.sync_info
            if si is not None and len(si.on_wait) > max_waits:
                waits = list(si.on_wait)
                keep = waits[-max_waits:]
                for w in waits[:-max_waits]:
                    nop = mybir.InstNoOp(name=f"splitw-{ctr}", ins=[], outs=[])
                    ctr += 1
                    nop.engine = inst.engine
                    nop.sync_info = mybir.SyncInfo(on_wait=[w], on_update=[])
                    new_list.append(nop)
                inst.sync_info = mybir.SyncInfo(
                    on_wait=keep, on_update=list(si.on_update)
                )
                changed = True
            new_list.append(inst)
        if changed:
            bb.instructions = new_list


def build_kernel() -> bass.Bass:
    nc = bass.Bass()
    # natural layouts: wq/wk rows = output feature e (the M contraction dim)
    wq = nc.dram_tensor("wq", [D, D], BF16, kind="ExternalInput")
    wk = nc.dram_tensor("wk", [D, D], BF16, kind="ExternalInput")
    xtq = nc.dram_tensor("xtq", [D, NQ], BF16, kind="ExternalInput")
    xt = nc.dram_tensor("xt", [D, N], BF16, kind="ExternalInput")
    x_nat = nc.dram_tensor("x", [N, D], BF16, kind="ExternalInput")
    out = nc.dram_tensor("out", [NQ, D], F32, kind="ExternalOutput")

    wq_r = wq.rearrange("(c p) i -> p c i", p=P)
    wk_r = wk.rearrange("(c p) j -> p c j", p=P)
    xtq_r = xtq.rearrange("(c p) n -> p c n", p=P)
    xt_r = xt.rearrange("(c p) n -> p c n", p=P)

    with SplitDrainTileContext(nc) as tc:
        with (
            tc.tile_pool(name="psum", bufs=7, space="PSUM") as pp,
            tc.tile_pool(name="psum_s", bufs=1, space="PSUM") as pps,
            tc.tile_pool(name="persist", bufs=1) as persist,
        ):
            ones_f32 = persist.tile([P, 1], F32, name="ones_f32", tag="ones32")
            nc.vector.memset(ones_f32, 1.0)
            ones_b = persist.tile([P, 1], BF16, name="ones_b", tag="ones")
            nc.scalar.copy(ones_b, ones_f32)

            m_sb = persist.tile([P, DC, D], BF16, name="m_sb", tag="m_sb")
            qt_sb = persist.tile([P, DC, NQ], BF16, name="qt_sb", tag="qt_sb")
            xt_sb = persist.tile([P, DC, N], BF16, name="xt_sb", tag="xt_sb")

            # ---------------- phase M + Q: projections ---------------------
            with (
                tc.tile_pool(name="wpool", bufs=1) as wpool,
                tc.tile_pool(name="xqp", bufs=2) as xqp,
            ):
                # wk first: M's inner loop streams wk halves, so the first
                # matmul needs all of wk but only the first wq column chunk
                wk_sb = wpool.tile([P, DC, D], BF16, name="wk_sb", tag="wk")
                for h in range(2):
                    nc.sync.dma_start(
                        out=wk_sb[:, h * DC // 2:(h + 1) * DC // 2, :],
                        in_=wk_r[:, h * DC // 2:(h + 1) * DC // 2, :],
                    )
                wq_sb = wpool.tile([P, DC, D], BF16, name="wq_sb", tag="wq")
                for h in range(4):
                    nc.sync.dma_start(
                        out=wq_sb[:, :, h * D // 4:(h + 1) * D // 4],
                        in_=wq_r[:, :, h * D // 4:(h + 1) * D // 4],
                    )
                # resident x^T (8MB bf16): needed only once phase A starts
                for h in range(DC):
                    nc.sync.dma_start(out=xt_sb[:, h, :], in_=xt_r[:, h, :])

                # phase M: M[i,j] = sum_e Wq[e,i] Wk[e,j]
                for ic in range(DC):
                    for jh in range(2):
                        ps = pp.tile([P, QS], F32, name="ps_m", tag="bank")
                        for ec in range(DC):
                            nc.tensor.matmul(
                                ps,
                                wq_sb[:, ec, ic * P:(ic + 1) * P],
                                wk_sb[:, ec, jh * QS:(jh + 1) * QS],
                                start=(ec == 0),
                                stop=(ec == DC - 1),
                            )
                        nc.scalar.copy(m_sb[:, ic, jh * QS:(jh + 1) * QS], ps)

                # phase Q: q'T[j, n] = sum_i M[i, j] x_q[n, i]
                for qb in range(NQS):
                    xblk = xqp.tile([P, DC, QS], BF16, name="xblk", tag="xblk")
                    nc.sync.dma_start(
                        out=xblk, in_=xtq_r[:, :, qb * QS:(qb + 1) * QS]
                    )
                    for jc in range(DC):
                        ps = pp.tile([P, QS], F32, name="ps_q", tag="bank")
                        for ic in range(DC):
                            nc.tensor.matmul(
                                ps,
                                m_sb[:, ic, jc * P:(jc + 1) * P],
                                xblk[:, ic, :],
                                start=(ic == 0),
                                stop=(ic == DC - 1),
                            )
                        nc.scalar.copy(
                            qt_sb[:, jc, qb * QS:(qb + 1) * QS], ps
                        )

            # ---------------- main loop ----------------------------------
            with (
                tc.tile_pool(name="ptp", bufs=1) as ptp,
                tc.tile_pool(name="xbp", bufs=6) as xbp,
                tc.tile_pool(name="outp", bufs=4) as outp,
                tc.tile_pool(name="smallp", bufs=2) as smallp,
            ):
                for qs in range(NQS):
                    q0 = qs * QS

                    # phase A: transposed score chunks -> exp -> pt (bf16)
                    pt_tiles = []
                    for nk in range(NCH):
                        ps = pp.tile([P, QS], F32, name="ps_sc", tag="bank")
                        for jc in range(DC):
                            nc.tensor.matmul(
                                ps,
                                xt_sb[:, jc, nk * P:(nk + 1) * P],
                                qt_sb[:, jc, q0:q0 + QS],
                                start=(jc == 0),
                                stop=(jc == DC - 1),
                            )
                        pt = ptp.tile([P, QS], BF16, name="pt", tag=f"pt{nk}")
                        nc.scalar.activation(pt, ps, EXP, scale=SCALE)
                        pt_tiles.append(pt)

                    # phase B: two 4-bank output groups (d-halves); the eh=0
                    # group also accumulates softmax denominators via f=1
                    # matmuls sharing the pt stationary
                    s_ps = pps.tile([P, NSUB], F32, name="s_ps", tag="s_bank")
                    recip = None
                    for eh in range(2):
                        ps_o = [
                            pp.tile([P, QS], F32, name="ps_o", tag="bank")
                            for _ in range(NSUB)
                        ]
                        for nk in range(NCH):
                            xc = xbp.tile([P, QS], BF16, name="xc", tag="xc")
                            nc.sync.dma_start(
                                out=xc,
                                in_=x_nat[nk * P:(nk + 1) * P,
                                          eh * QS:(eh + 1) * QS],
                            )
                            for qsub in range(NSUB):
                                lhsT = pt_tiles[nk][:, qsub * P:(qsub + 1) * P]
                                if eh == 0:
                                    # start=True zeroes the WHOLE bank, so
                                    # only the first write may carry it; the
                                    # other qsub columns accumulate onto the
                                    # zeroed bank
                                    nc.tensor.matmul(
                                        s_ps[:, qsub:qsub + 1],
                                        lhsT,
                                        ones_b,
                                        start=(nk == 0 and qsub == 0),
                                        stop=(nk == NCH - 1),
                                        skip_group_check=True,
                                    )
                                nc.tensor.matmul(
                                    ps_o[qsub],
                                    lhsT,
                                    xc,
                                    start=(nk == 0),
                                    stop=(nk == NCH - 1),
                                )
                        if eh == 0:
                            s_sb = smallp.tile(
                                [P, NSUB], F32, name="s_sb", tag="s_sb"
                            )
                            nc.scalar.copy(s_sb, s_ps)
                            recip = smallp.tile(
                                [P, NSUB], F32, name="recip", tag="recip"
                            )
                            nc.vector.reciprocal(recip, s_sb)
                        for qsub in range(NSUB):
                            o_sb = outp.tile(
                                [P, QS], F32, name="o_sb", tag="o_sb"
                            )
                            nc.vector.tensor_scalar_mul(
                                o_sb, ps_o[qsub], recip[:, qsub:qsub + 1]
                            )
                            nc.sync.dma_start(
                                out=out[
                                    q0 + qsub * P:q0 + (qsub + 1) * P,
                                    eh * QS:(eh + 1) * QS,
                                ],
                                in_=o_sb,
                            )
    _split_multiwaits(nc)
    return nc


def _make_in_maps(x, Wq, Wk):
    import ml_dtypes

    bf16 = ml_dtypes.bfloat16
    x = np.asarray(x, dtype=np.float32)
    wq_b = np.ascontiguousarray(np.asarray(Wq, dtype=np.float32), dtype=bf16)
    wk_b = np.ascontiguousarray(np.asarray(Wk, dtype=np.float32), dtype=bf16)
    in_maps = []
    for c in range(N_CORES):
        b, h = divmod(c, 2)
        xtb = np.ascontiguousarray(x[b].T).astype(bf16)
        in_maps.append(
            {
                "x": np.ascontiguousarray(x[b]).astype(bf16),
                "xt": xtb,
                "xtq": np.ascontiguousarray(xtb[:, h * NQ:(h + 1) * NQ]),
                "wq": wq_b,
                "wk": wk_b,
            }
        )
    return in_maps


_NC_CACHE = None
_RUNNER_CACHE = None


def _make_runner(nc):
    """Build the sharded PJRT callable once so repeated kernel() calls reuse
    the jit cache (mirrors concourse.bass2jax.run_bass_via_pjrt's multi-core
    branch)."""
    import jax
    from jax.experimental.shard_map import shard_map
    from jax.sharding import Mesh, PartitionSpec

    from concourse import bass2jax

    bass2jax.install_neuronx_cc_hook()

    partition_name = nc.partition_id_tensor.name if nc.partition_id_tensor else None
    in_names, out_names, out_avals, zero_outs = [], [], [], []
    for alloc in nc.m.functions[0].allocations:
        if not isinstance(alloc, mybir.MemoryLocationSet):
            continue
        name = alloc.memorylocations[0].name
        if alloc.kind == "ExternalInput":
            if name != partition_name:
                in_names.append(name)
        elif alloc.kind == "ExternalOutput":
            shape = tuple(alloc.tensor_shape)
            dtype = mybir.dt.np(alloc.dtype)
            out_names.append(name)
            out_avals.append(jax.core.ShapedArray(shape, dtype))
            zero_outs.append(np.zeros(shape, dtype))
    n_params = len(in_names)
    n_outs = len(out_avals)
    all_in_names = list(in_names) + list(out_names)
    if partition_name is not None:
        all_in_names.append(partition_name)
    donate = tuple(range(n_params, n_params + n_outs))

    def _body(*args):
        operands = list(args)
        if partition_name is not None:
            operands.append(bass2jax.partition_id_tensor())
        outs = bass2jax._bass_exec_p.bind(
            *operands,
            out_avals=tuple(out_avals),
            in_names=tuple(all_in_names),
            out_names=tuple(out_names),
            lowering_input_output_aliases=(),
            sim_require_finite=True,
            sim_require_nnan=True,
            nc=nc,
        )
        return tuple(outs)

    devices = jax.devices()[:N_CORES]
    mesh = Mesh(np.asarray(devices), ("core",))
    in_specs = (PartitionSpec("core"),) * (n_params + n_outs)
    out_specs = (PartitionSpec("core"),) * n_outs
    sharded = jax.jit(
        shard_map(
            _body, mesh=mesh, in_specs=in_specs, out_specs=out_specs,
            check_rep=False,
        ),
        donate_argnums=donate,
        keep_unused=True,
    )

    def run(in_maps):
        concat_in = [
            np.concatenate([np.asarray(m[nm]) for m in in_maps], axis=0)
            for nm in in_names
        ]
        concat_zeros = [
            np.zeros((N_CORES * z.shape[0], *z.shape[1:]), z.dtype)
            for z in zero_outs
        ]
        out_arrs = sharded(*concat_in, *concat_zeros)
        return [
            {
                nm: np.asarray(out_arrs[i]).reshape(
                    N_CORES, *out_avals[i].shape
                )[c]
                for i, nm in enumerate(out_names)
            }
            for c in range(N_CORES)
        ]

    return run


def kernel(x: np.ndarray, Wq: np.ndarray, Wk: np.ndarray) -> np.ndarray:
    global _NC_CACHE, _RUNNER_CACHE
    if _NC_CACHE is None:
        _NC_CACHE = build_kernel()
    nc = _NC_CACHE

    in_maps = _make_in_maps(x, Wq, Wk)

    results = None
    try:
        if _RUNNER_CACHE is None:
            _RUNNER_CACHE = _make_runner(nc)
        results = _RUNNER_CACHE(in_maps)
    except Exception:
        _RUNNER_CACHE = None
        results = None
    if results is None:
        # fallback: the supported (slower, per-call jit) path
        from concourse.bass_utils import run_bass_kernel_spmd

        results = run_bass_kernel_spmd(
            nc, in_maps, core_ids=list(range(N_CORES))
        ).results

    outv = np.empty((B, N, D), dtype=np.float32)
    for c in range(N_CORES):
        b, h = divmod(c, 2)
        outv[b, h * NQ:(h + 1) * NQ, :] = results[c]["out"]
    return outv
